# revision 1
# baseline (speedup 1.0000x reference)
"""Trainium2 (8 NeuronCores) Bass kernel for the GNN message-passing module.

v4 rewrite of the fast path (block-diagonal adjacency + bond_index):

  - All gathers are non-transposed (pair-major layout): descriptor writes
    are 256B contiguous per partition instead of 2B x 128 partitions.
  - Block-diagonal adjacency: the MPS stage only needs the core's own
    128x128 adjacency block and own-atom embeddings (8x less work than
    computing contri for all 1024 atoms).
  - inb2[pa, n, c] layout (pa = own atom, n = in-molecule neighbor,
    c = feature): ssq/w/tmp all computed with DVE strided reduces and one
    TensorE molecule-segment reduction (sel matmul).
  - Cross-core coupling is only gwsum[c] = sum_a w[a, c].  Exchanged with
    raw remote SBUF DMA broadcasts (XOR-relative routing, one column per
    XOR offset) instead of a collective_compute: no NRT CC-stream init
    barrier, no Mesh algorithm latency.  comm="cc" falls back to a real
    AllReduce.
  - Finale MLP is transpose-free: out[j,m] = sum_d W[d,j] x[d,m] keeps
    everything feature-major; relu+bias fused into scalar activations.
"""

import os
import sys

for _p in ("/opt/trn_rl_repo",):
    if _p not in sys.path:
        sys.path.insert(0, _p)

import numpy as np
import ml_dtypes

# Some images lack antenv.axon_hooks; bass_utils imports it unguarded when
# trace=True. Provide a shim so the import never crashes.
try:
    import antenv.axon_hooks  # noqa: F401
except ImportError:
    import types as _types

    import antenv as _antenv

    _m = _types.ModuleType("antenv.axon_hooks")
    _m._hook = None

    def _set_hook(h):
        _m._hook = h

    def _get_hook():
        return _m._hook

    _m.set_axon_ntff_profile_hook = _set_hook
    _m.get_axon_ntff_profile_hook = _get_hook
    sys.modules["antenv.axon_hooks"] = _m
    _antenv.axon_hooks = _m

import concourse.bacc as bacc
import concourse.mybir as mybir
import concourse.tile as tile
from concourse.bass_utils import run_bass_kernel_spmd

BF16 = ml_dtypes.bfloat16

NCORES = 8
N = 1024          # total atoms
DIM = 128
B = 64            # molecules
NA = 16           # atoms per molecule
APC = N // NCORES  # atoms per core = 128
MPC = B // NCORES  # molecules per core = 8
NIB = APC * NA     # in-block bond indices per core = 2048
N_FP = 10000
N_BOND = 10000
EPS = 1e-12

F32 = mybir.dt.float32
BF = mybir.dt.bfloat16
I16 = mybir.dt.int16
AOT = mybir.AluOpType
AFT = mybir.ActivationFunctionType


def _wrap_idx16(flat):
    """SWDGE dma_gather index layout: idx i -> partition i%16, col i//16,
    replicated across the 8 Q7 cores (rows 16..127)."""
    flat = np.ascontiguousarray(flat.astype(np.int16))
    n = flat.shape[0]
    assert n % 16 == 0
    w = flat.reshape(n // 16, 16).T  # [16, n//16]
    return np.tile(w, (8, 1))        # [128, n//16]


def build_v4(comm="cc"):
    nc = bacc.Bacc("TRN2", target_bir_lowering=False, debug=False,
                   num_devices=NCORES)

    e_bond = nc.declare_dram_parameter("e_bond", [N_BOND, DIM], BF, False)
    e_fp = nc.declare_dram_parameter("e_fp", [N_FP, DIM], BF, False)
    fpi = nc.declare_dram_parameter("fpi", [128, APC // 16], I16, False)
    ibx2 = nc.declare_dram_parameter("ibx2", [128, NIB // 16], I16, False)
    atT = nc.declare_dram_parameter("atT", [128, 128], BF, False)
    wfp = nc.declare_dram_parameter("wfp", [DIM, DIM], BF, False)
    bfp = nc.declare_dram_parameter("bfp", [1, DIM], BF, False)
    w0 = nc.declare_dram_parameter("w0", [DIM, DIM], F32, False)
    b0T = nc.declare_dram_parameter("b0T", [DIM, 1], F32, False)
    w1 = nc.declare_dram_parameter("w1", [DIM, DIM], F32, False)
    b1T = nc.declare_dram_parameter("b1T", [DIM, 1], F32, False)
    wp = nc.declare_dram_parameter("wp", [DIM, 1], F32, False)
    bp = nc.declare_dram_parameter("bp", [1, 1], F32, False)
    idbf = nc.declare_dram_parameter("idbf", [128, 128], BF, False)
    idf = nc.declare_dram_parameter("idf", [128, 128], F32, False)
    onesbf = nc.declare_dram_parameter("onesbf", [1, 128], BF, False)
    onecol = nc.declare_dram_parameter("onecol", [128, 1], BF, False)
    onecolf = nc.declare_dram_parameter("onecolf", [128, 1], F32, False)
    selbf = nc.declare_dram_parameter("selbf", [128, MPC], BF, False)
    t0sqk = nc.declare_dram_parameter("t0sqk", [128, DIM], F32, False)
    t0rep8 = nc.declare_dram_parameter("t0rep8", [MPC, DIM], F32, False)
    out = nc.declare_dram_parameter("out", [1, MPC], F32, True)

    scratch_mn = nc.dram_tensor("scratch_mn", [APC, DIM], F32)
    if comm == "cc":
        cc_in = nc.dram_tensor("cc_in", [128, 1], F32)
        cc_out = nc.dram_tensor("cc_out", [128, 1], F32, addr_space="Shared")

    if comm == "rdma":
        # Allocated OUTSIDE the TileContext: the rsem-gated tail is raw
        # bass emitted after the context (the single-core tile scheduling
        # sim cannot model remote sem increments and would deadlock), and
        # tile's exit sem-clear must not touch these.
        rsem = nc.alloc_semaphore("gw_rsem")
        lsem = nc.alloc_semaphore("gw_lsem")
        tails = [nc.alloc_semaphore(f"tail{i}") for i in range(8)]

    with tile.TileContext(nc) as tc:
        with (
            tc.tile_pool(name="const", bufs=1) as cp,
            tc.tile_pool(name="small", bufs=2) as smp,
            tc.tile_pool(name="psB", bufs=2, space="PSUM") as psB,
            tc.tile_pool(name="psT", bufs=2, space="PSUM") as psT,
        ):
            if comm == "rdma":
                prep_sem = nc.alloc_semaphore("gw_prep")
                wsum_sem = nc.alloc_semaphore("gw_src")
                # receive buffer: written ONLY by remote DMA (col d holds
                # the wsum of the peer at XOR offset d); summed after
                # rsem==16 (2 increments per arriving broadcast x 8 peers).
                allw = cp.tile([128, MPC], F32, tag="allw")
            wsum_s = cp.tile([128, 1], F32, tag="wsum_s")

            # ---- index DMAs first, then gathers ------------------------
            ibx2_s = cp.tile([128, NIB // 16], I16, tag="ibx2_s")
            nc.sync.dma_start(ibx2_s[:], ibx2[:])
            fpi_s = cp.tile([128, APC // 16], I16, tag="fpi_s")
            nc.sync.dma_start(fpi_s[:], fpi[:])

            # inb2[pa, n, c] = E_bond[bond_index[own atom pa, nbr n]]
            # Split in two halves (n 0..7 / 8..15) so the first half's ssq
            # partial reduce overlaps the second half's DMA.
            inb2 = cp.tile([128, NA, DIM], BF, tag="inb2")
            nh = NIB // 2
            nc.gpsimd.dma_gather(
                out_ap=inb2[:, :NA // 2, :], in_ap=e_bond[:],
                idxs_ap=ibx2_s[:, :nh // 16], num_idxs=nh, num_idxs_reg=nh,
                elem_size=DIM, transpose=False, single_packet=False)
            # mps0own[a, c] for own 128 atoms
            mps0own = cp.tile([128, 1, DIM], BF, tag="mps0own")
            nc.gpsimd.dma_gather(
                out_ap=mps0own[:], in_ap=e_fp[:], idxs_ap=fpi_s[:],
                num_idxs=APC, num_idxs_reg=APC, elem_size=DIM,
                transpose=False, single_packet=False)
            nc.gpsimd.dma_gather(
                out_ap=inb2[:, NA // 2:, :], in_ap=e_bond[:],
                idxs_ap=ibx2_s[:, nh // 16:], num_idxs=nh, num_idxs_reg=nh,
                elem_size=DIM, transpose=False, single_packet=False)

            if comm == "rdma":
                # prepare the 8 single-dest broadcasts early (desc-gen only;
                # data is read at trigger time).  Sender with tpb t, slot d
                # -> dest tpb t^d, writing column d; every receiver's 8
                # columns are a permutation of all 8 cores' wsum vectors.
                for d in range(NCORES):
                    rdests = [None] * NCORES
                    rdests[d] = (0, d)
                    nc.gpsimd.remote_dma_broadcast(
                        out_ap=allw[:, d:d + 1], in_ap=wsum_s[:],
                        remote_sem=rsem, local_sem=lsem,
                        rdests=rdests,
                    ).then_inc(prep_sem, 1)

            # ---- constants ---------------------------------------------
            wfp_s = cp.tile([DIM, DIM], BF, tag="wfp_s")
            nc.sync.dma_start(wfp_s[:], wfp[:])
            bfp_s = cp.tile([1, DIM], BF, tag="bfp_s")
            nc.sync.dma_start(bfp_s[:], bfp[:])
            idbf_s = cp.tile([128, 128], BF, tag="idbf_s")
            nc.sync.dma_start(idbf_s[:], idbf[:])
            idf_s = cp.tile([128, 128], F32, tag="idf_s")
            nc.sync.dma_start(idf_s[:], idf[:])
            onesbf_s = cp.tile([1, 128], BF, tag="onesbf_s")
            nc.sync.dma_start(onesbf_s[:], onesbf[:])
            onecol_s = cp.tile([128, 1], BF, tag="onecol_s")
            nc.sync.dma_start(onecol_s[:], onecol[:])
            onecolf_s = cp.tile([128, 1], F32, tag="onecolf_s")
            nc.sync.dma_start(onecolf_s[:], onecolf[:])
            selbf_s = cp.tile([128, MPC], BF, tag="selbf_s")
            nc.sync.dma_start(selbf_s[:], selbf[:])
            atT_s = cp.tile([128, 128], BF, tag="atT_s")
            nc.sync.dma_start(atT_s[:], atT[:])
            t0sqk_s = cp.tile([128, DIM], F32, tag="t0sqk_s")
            nc.sync.dma_start(t0sqk_s[:], t0sqk[:])
            t0rep8_s = cp.tile([MPC, DIM], F32, tag="t0rep8_s")
            nc.sync.dma_start(t0rep8_s[:], t0rep8[:])
            w0_s = cp.tile([DIM, DIM], F32, tag="w0_s")
            nc.sync.dma_start(w0_s[:], w0[:])
            b0T_s = cp.tile([DIM, 1], F32, tag="b0T_s")
            nc.sync.dma_start(b0T_s[:], b0T[:])
            w1_s = cp.tile([DIM, DIM], F32, tag="w1_s")
            nc.sync.dma_start(w1_s[:], w1[:])
            b1T_s = cp.tile([DIM, 1], F32, tag="b1T_s")
            nc.sync.dma_start(b1T_s[:], b1T[:])
            wp_s = cp.tile([DIM, 1], F32, tag="wp_s")
            nc.sync.dma_start(wp_s[:], wp[:])
            bp_s = cp.tile([1, 1], F32, tag="bp_s")
            nc.sync.dma_start(bp_s[:], bp[:])

            # ---- MPS stage (own molecules only; A is block-diagonal) ---
            m0T_ps = psB.tile([128, 128], BF, tag="psbf")
            nc.tensor.transpose(m0T_ps[:], mps0own[:, 0, :], idbf_s[:])
            mps0ownT = cp.tile([128, 128], BF, tag="mps0ownT")  # [c, a]
            nc.vector.tensor_copy(mps0ownT[:], m0T_ps[:])

            cps = psB.tile([128, DIM], F32, tag="ps")
            nc.tensor.matmul(cps[:], lhsT=mps0ownT[:], rhs=wfp_s[:],
                             start=True, stop=False)
            nc.tensor.matmul(cps[:], lhsT=onesbf_s[:], rhs=bfp_s[:],
                             start=False, stop=True)
            contri_s = cp.tile([128, DIM], BF, tag="contri_s")  # [a, c']
            nc.scalar.activation(out=contri_s[:], in_=cps[:], func=AFT.Relu)

            mps_ps = psB.tile([128, DIM], F32, tag="ps")
            nc.tensor.matmul(mps_ps[:], lhsT=atT_s[:], rhs=contri_s[:],
                             start=True, stop=False)
            nc.tensor.matmul(mps_ps[:], lhsT=mps0ownT[:], rhs=idbf_s[:],
                             start=False, stop=True)
            mps_own = cp.tile([128, DIM], F32, tag="mps_own")  # [a, c]
            nc.vector.tensor_copy(mps_own[:], mps_ps[:])

            # l2 normalize rows (free axis)
            nsq = smp.tile([128, 1], F32, tag="nsq")
            nscr = smp.tile([128, DIM], F32, tag="nscr")
            nc.scalar.activation(out=nscr[:], in_=mps_own[:],
                                 func=AFT.Square, accum_out=nsq[:])
            nrm = smp.tile([128, 1], F32, tag="nrm")
            nc.scalar.sqrt(nrm[:], nsq[:])
            nrm2 = smp.tile([128, 1], F32, tag="nrm2")
            nc.vector.tensor_scalar_max(nrm2[:], nrm[:], EPS)
            inv = smp.tile([128, 1], F32, tag="inv")
            nc.vector.reciprocal(inv[:], nrm2[:])
            mps_n = cp.tile([128, DIM], F32, tag="mps_n")  # [a, c]
            nc.vector.tensor_scalar_mul(mps_n[:], mps_own[:], inv[:])

            # refold mps_n [128, c] -> mpsn2 [m, (n c)] via DRAM roundtrip
            nc.sync.dma_start(scratch_mn[:], mps_n[:])
            mpsn2 = cp.tile([MPC, NA * DIM], F32, tag="mpsn2")
            nc.sync.dma_start(
                mpsn2[:],
                scratch_mn[:].rearrange("(m r) c -> m (r c)", m=MPC))

            # ---- ssq / w ----------------------------------------------
            # partial ssq of gather half A (overlaps half B's DMA), with the
            # analytic off-block term folded in early; half B's chain is the
            # only post-gather work on the AllReduce trigger path.
            sq = smp.tile([128, NA, DIM], BF, tag="sq")
            nha = NA // 2
            nc.vector.tensor_mul(
                sq[:, :nha, :].rearrange("p n c -> p (n c)"),
                inb2[:, :nha, :].rearrange("p n c -> p (n c)"),
                inb2[:, :nha, :].rearrange("p n c -> p (n c)"))
            ssqa = smp.tile([128, DIM], F32, tag="ssqa")
            nc.vector.reduce_sum(
                ssqa.rearrange("p (c o) -> p c o", o=1),
                sq[:, :nha, :].rearrange("p n c -> p c n"),
                axis=mybir.AxisListType.X)
            ssqa2 = smp.tile([128, DIM], F32, tag="ssqa2")
            nc.vector.tensor_add(ssqa2[:], ssqa[:], t0sqk_s[:])
            nc.vector.tensor_mul(
                sq[:, nha:, :].rearrange("p n c -> p (n c)"),
                inb2[:, nha:, :].rearrange("p n c -> p (n c)"),
                inb2[:, nha:, :].rearrange("p n c -> p (n c)"))
            ssqb = smp.tile([128, DIM], F32, tag="ssqb")
            nc.vector.reduce_sum(
                ssqb.rearrange("p (c o) -> p c o", o=1),
                sq[:, nha:, :].rearrange("p n c -> p c n"),
                axis=mybir.AxisListType.X)
            ssq = smp.tile([128, DIM], F32, tag="ssq")
            nc.vector.tensor_add(ssq[:], ssqa2[:], ssqb[:])
            st = smp.tile([128, DIM], F32, tag="st")
            nc.scalar.sqrt(st[:], ssq[:])
            # no max(st, eps): ssq >= 1008*T0[c]^2 > 0 always.
            sti = smp.tile([128, DIM], F32, tag="sti")
            nc.vector.reciprocal(sti[:], st[:])
            w_s = cp.tile([128, DIM], F32, tag="w_s")    # [a, c]
            nc.vector.tensor_mul(w_s[:], mps_n[:], sti[:])

            # ---- wsum (column) + exchange ------------------------------
            # fp32 matmul straight from w_s: the bf16 copy of w (for the
            # wprod/tmp2 stage) moves off the trigger path.
            ws_ps = psB.tile([128, 1], F32, tag="ps")
            nc.tensor.matmul(ws_ps[:], lhsT=w_s[:], rhs=onecolf_s[:],
                             start=True, stop=True)
            wbf = cp.tile([128, DIM], BF, tag="wbf")
            nc.vector.tensor_copy(wbf[:], w_s[:])
            ci = nc.vector.tensor_copy(wsum_s[:], ws_ps[:])
            if comm == "rdma":
                ci.then_inc(wsum_sem, 1)
                nc.gpsimd.wait_ge(prep_sem, NCORES)
                nc.gpsimd.wait_ge(wsum_sem, 1)
                nc.gpsimd.trigger_dma(count=NCORES)
            else:
                nc.sync.dma_start(cc_in[:], wsum_s[:])
                nc.gpsimd.collective_compute(
                    "AllReduce", AOT.add,
                    replica_groups=[list(range(NCORES))],
                    ins=[cc_in[:]], outs=[cc_out[:]])

            # ---- tmp2 via molecule-segment matmul ----------------------
            wprod = cp.tile([128, NA, DIM], BF, tag="wprod")
            nc.vector.tensor_mul(
                wprod[:], inb2[:],
                wbf[:].rearrange("p (o c) -> p o c", o=1)
                      .broadcast_to([128, NA, DIM]))
            wprod_f = wprod.rearrange("p n c -> p (n c)")
            tmp2_s = cp.tile([MPC, NA * DIM], F32, tag="tmp2_s")
            for q in range(4):
                tp = psT.tile([MPC, 512], F32, tag="ps")
                nc.tensor.matmul(tp[:], lhsT=selbf_s[:],
                                 rhs=wprod_f[:, q * 512:(q + 1) * 512],
                                 start=True, stop=True)
                nc.scalar.activation(out=tmp2_s[:, q * 512:(q + 1) * 512],
                                     in_=tp[:], func=AFT.Copy)
            wm_ps = psT.tile([MPC, DIM], F32, tag="ps")
            nc.tensor.matmul(wm_ps[:], lhsT=selbf_s[:], rhs=wbf[:],
                             start=True, stop=True)
            wmol_s = smp.tile([MPC, DIM], F32, tag="wmol_s")
            nc.scalar.activation(out=wmol_s[:], in_=wm_ps[:], func=AFT.Copy)

            # ---- tn parts (all pre-exchange) ---------------------------
            fp2 = smp.tile([MPC, NA * DIM], F32, tag="fp2")
            nc.vector.tensor_mul(fp2[:], tmp2_s[:], mpsn2[:])
            tn0 = smp.tile([MPC, DIM], F32, tag="tn0")
            nc.vector.reduce_sum(
                tn0.rearrange("p (c o) -> p c o", o=1),
                fp2.rearrange("p (n c) -> p n c", c=DIM).transpose([0, 2, 1]),
                axis=mybir.AxisListType.X)
            vmol = smp.tile([MPC, DIM], F32, tag="vmol")
            nc.vector.reduce_sum(
                vmol.rearrange("p (c o) -> p c o", o=1),
                mpsn2.rearrange("p (n c) -> p n c", c=DIM)
                     .transpose([0, 2, 1]),
                axis=mybir.AxisListType.X)
            u = smp.tile([MPC, DIM], F32, tag="u")
            nc.vector.tensor_mul(u[:], vmol[:], t0rep8_s[:])
            wu = smp.tile([MPC, DIM], F32, tag="wu")
            nc.vector.tensor_mul(wu[:], wmol_s[:], u[:])
            tnb = smp.tile([MPC, DIM], F32, tag="tnb")
            nc.vector.scalar_tensor_tensor(
                out=tnb[:], in0=wu[:], scalar=-1.0, in1=tn0[:],
                op0=AOT.mult, op1=AOT.add)

            tb_ps = psB.tile([128, MPC], F32, tag="ps")
            nc.tensor.transpose(tb_ps[:], tnb[:], idf_s[:MPC, :MPC])
            tnbT = cp.tile([128, MPC], F32, tag="tnbT")
            nc.vector.tensor_copy(tnbT[:], tb_ps[:])
            u_ps = psB.tile([128, MPC], F32, tag="ps")
            nc.tensor.transpose(u_ps[:], u[:], idf_s[:MPC, :MPC])
            uT = cp.tile([128, MPC], F32, tag="uT")
            nc.scalar.activation(out=uT[:], in_=u_ps[:], func=AFT.Copy)

            # tail tiles (persistent addresses; rdma uses them raw)
            gw = cp.tile([128, 1], F32, tag="gw")
            tnT = cp.tile([128, MPC], F32, tag="tnT")
            x0T = cp.tile([128, MPC], F32, tag="x0T")
            x1T = cp.tile([128, MPC], F32, tag="x1T")
            y_s = cp.tile([1, MPC], F32, tag="y_s")
            xps = psT.tile([128, MPC], F32, tag="xps")
            x0_ps = x1_ps = xps
            y_ps = xps[:1, :]

            if comm == "cc":
                # ---- receive gw, assemble tn, tiny MLP (in-tile) -------
                nc.sync.dma_start(gw[:], cc_out[:])
                nc.vector.scalar_tensor_tensor(
                    out=tnT[:], in0=uT[:], scalar=gw[:], in1=tnbT[:],
                    op0=AOT.mult, op1=AOT.add)
                nc.tensor.matmul(x0_ps[:], lhsT=w0_s[:], rhs=tnT[:],
                                 start=True, stop=True)
                nc.scalar.activation(out=x0T[:], in_=x0_ps[:],
                                     func=AFT.Relu, bias=b0T_s[:])
                nc.tensor.matmul(x1_ps[:], lhsT=w1_s[:], rhs=x0T[:],
                                 start=True, stop=True)
                nc.scalar.activation(out=x1T[:], in_=x1_ps[:],
                                     func=AFT.Relu, bias=b1T_s[:])
                nc.tensor.matmul(y_ps[:], lhsT=wp_s[:], rhs=x1T[:],
                                 start=True, stop=True)
                nc.vector.tensor_scalar_add(y_s[:], y_ps[:], bp_s[:])
                nc.sync.dma_start(out[:], y_s[:])

    if comm == "rdma":
        # ---- raw tail (post-tile): wait for all 8 wsum columns ---------
        nc.vector.wait_ge(rsem, 16)
        nc.vector.reduce_sum(gw[:], allw[:], axis=mybir.AxisListType.X)
        nc.vector.scalar_tensor_tensor(
            out=tnT[:], in0=uT[:], scalar=gw[:], in1=tnbT[:],
            op0=AOT.mult, op1=AOT.add).then_inc(tails[0], 1)
        nc.tensor.wait_ge(tails[0], 1)
        nc.tensor.matmul(x0_ps[:], lhsT=w0_s[:], rhs=tnT[:],
                         start=True, stop=True).then_inc(tails[1], 1)
        nc.vector.wait_ge(tails[1], 1)
        nc.vector.tensor_scalar(
            x0T[:], x0_ps[:], b0T_s[:], 0.0,
            op0=AOT.add, op1=AOT.max).then_inc(tails[2], 1)
        nc.tensor.wait_ge(tails[2], 1)
        nc.tensor.matmul(x1_ps[:], lhsT=w1_s[:], rhs=x0T[:],
                         start=True, stop=True).then_inc(tails[3], 1)
        nc.vector.wait_ge(tails[3], 1)
        nc.vector.tensor_scalar(
            x1T[:], x1_ps[:], b1T_s[:], 0.0,
            op0=AOT.add, op1=AOT.max).then_inc(tails[4], 1)
        nc.tensor.wait_ge(tails[4], 1)
        nc.tensor.matmul(y_ps[:], lhsT=wp_s[:], rhs=x1T[:],
                         start=True, stop=True).then_inc(tails[5], 1)
        nc.vector.wait_ge(tails[5], 1)
        nc.vector.tensor_scalar_add(
            y_s[:], y_ps[:], bp_s[:]).then_inc(tails[6], 1)
        nc.sync.wait_ge(tails[6], 1)
        nc.sync.dma_start(out[:], y_s[:]).then_inc(tails[7], 16)
        nc.sync.wait_ge(tails[7], 16)
        # zero the cross-run sems so a re-execution starts clean (all 8
        # peers' increments are consumed by the rsem wait above)
        nc.gpsimd.wait_ge(tails[7], 16)
        nc.gpsimd.sem_clear(range(rsem.num, rsem.num + 1))
        for s in tails:
            nc.gpsimd.sem_clear(range(s.num, s.num + 1))
        nc.gpsimd.sem_clear(range(lsem.num, lsem.num + 1))

    nc.compile()
    return nc



NCH = 8           # compute groups per core (generic full path)
ACH = APC // NCH  # a-rows per group = 16
GCALLS = 4        # gather calls per group
GIDX = ACH * N // GCALLS  # 4096 indices per gather call
DVE_J = 7


def build_nc(stage="full"):
    nc = bacc.Bacc("TRN2", target_bir_lowering=False, debug=False,
                   num_devices=NCORES)

    e_bond = nc.declare_dram_parameter("e_bond", [N_BOND, DIM], BF, False)
    e_fp = nc.declare_dram_parameter("e_fp", [N_FP, DIM], BF, False)
    fpi_all = nc.declare_dram_parameter("fpi_all", [128, N // 16], I16, False)
    fpi_own = nc.declare_dram_parameter("fpi_own", [128, APC // 16], I16, False)
    bidx = nc.declare_dram_parameter("bidx", [128, APC * N // 16], I16, False)
    at = nc.declare_dram_parameter("at", [N, APC], BF, False)
    wfp = nc.declare_dram_parameter("wfp", [DIM, DIM], BF, False)
    bfp = nc.declare_dram_parameter("bfp", [1, DIM], BF, False)
    w0 = nc.declare_dram_parameter("w0", [DIM, DIM], F32, False)
    b0 = nc.declare_dram_parameter("b0", [1, DIM], F32, False)
    w1 = nc.declare_dram_parameter("w1", [DIM, DIM], F32, False)
    b1 = nc.declare_dram_parameter("b1", [1, DIM], F32, False)
    wp = nc.declare_dram_parameter("wp", [DIM, 1], F32, False)
    bp = nc.declare_dram_parameter("bp", [1, 1], F32, False)
    idbf = nc.declare_dram_parameter("idbf", [128, 128], BF, False)
    idf = nc.declare_dram_parameter("idf", [128, 128], F32, False)
    onesbf = nc.declare_dram_parameter("onesbf", [1, 128], BF, False)
    onesf = nc.declare_dram_parameter("onesf", [1, 128], F32, False)
    sel = nc.declare_dram_parameter("sel", [128, MPC], F32, False)
    out = nc.declare_dram_parameter("out", [MPC, 1], F32, True)

    cc_in = nc.dram_tensor("cc_in", [N, DIM], F32)
    cc_out = nc.dram_tensor("cc_out", [APC, DIM], F32)

    with tile.TileContext(nc) as tc:
        with (
            tc.tile_pool(name="const", bufs=1) as cp,
            tc.tile_pool(name="slab", bufs=2) as slabp,
            tc.tile_pool(name="scr", bufs=2) as scrp,
            tc.tile_pool(name="small", bufs=2) as smp,
            tc.tile_pool(name="diag", bufs=4) as diagp,
            tc.tile_pool(name="psA", bufs=1, space="PSUM") as psA,
            tc.tile_pool(name="psB", bufs=3, space="PSUM") as psB,
        ):
            # ---- constants to SBUF -------------------------------------
            wfp_s = cp.tile([DIM, DIM], BF, tag="wfp_s")
            nc.sync.dma_start(wfp_s[:], wfp[:])
            idbf_s = cp.tile([128, 128], BF, tag="idbf_s")
            nc.sync.dma_start(idbf_s[:], idbf[:])
            idf_s = cp.tile([128, 128], F32, tag="idf_s")
            nc.sync.dma_start(idf_s[:], idf[:])
            onesbf_s = cp.tile([1, 128], BF, tag="onesbf_s")
            nc.sync.dma_start(onesbf_s[:], onesbf[:])
            onesf_s = cp.tile([1, 128], F32, tag="onesf_s")
            nc.sync.dma_start(onesf_s[:], onesf[:])
            bfp_s = cp.tile([1, DIM], BF, tag="bfp_s")
            nc.sync.dma_start(bfp_s[:], bfp[:])
            w0_s = cp.tile([DIM, DIM], F32, tag="w0_s")
            nc.sync.dma_start(w0_s[:], w0[:])
            b0_s = cp.tile([1, DIM], F32, tag="b0_s")
            nc.sync.dma_start(b0_s[:], b0[:])
            w1_s = cp.tile([DIM, DIM], F32, tag="w1_s")
            nc.sync.dma_start(w1_s[:], w1[:])
            b1_s = cp.tile([1, DIM], F32, tag="b1_s")
            nc.sync.dma_start(b1_s[:], b1[:])
            wp_s = cp.tile([DIM, 1], F32, tag="wp_s")
            nc.sync.dma_start(wp_s[:], wp[:])
            bp_s = cp.tile([1, 1], F32, tag="bp_s")
            nc.sync.dma_start(bp_s[:], bp[:])
            sel_s = cp.tile([128, MPC], F32, tag="sel_s")
            nc.sync.dma_start(sel_s[:], sel[:])
            fpi_all_s = cp.tile([128, N // 16], I16, tag="fpi_all_s")
            nc.sync.dma_start(fpi_all_s[:], fpi_all[:])
            fpi_own_s = cp.tile([128, APC // 16], I16, tag="fpi_own_s")
            nc.sync.dma_start(fpi_own_s[:], fpi_own[:])
            bidx_s = cp.tile([128, APC * N // 16], I16, tag="bidx_s")
            nc.sync.dma_start(bidx_s[:], bidx[:])
            at_s = cp.tile([128, NCH, 128], BF, tag="at_s")
            for j in range(NCH):
                nc.sync.dma_start(at_s[:, j, :], at[j * 128:(j + 1) * 128, :])

            # ---- MPS stage ---------------------------------------------
            # mps0T: [c, b] bf16 for all 1024 atoms (replicated compute)
            mps0T = cp.tile([128, 1, N], BF, tag="mps0T")
            nc.gpsimd.dma_gather(
                out_ap=mps0T[:], in_ap=e_fp[:], idxs_ap=fpi_all_s[:],
                num_idxs=N, num_idxs_reg=N, elem_size=DIM, transpose=True,
                single_packet=False)
            # mps0 for own rows, [c, a_own]
            mps0oT = cp.tile([128, 1, APC], BF, tag="mps0oT")
            nc.gpsimd.dma_gather(
                out_ap=mps0oT[:], in_ap=e_fp[:], idxs_ap=fpi_own_s[:],
                num_idxs=APC, num_idxs_reg=APC, elem_size=DIM, transpose=True,
                single_packet=False)

            # contri[b, c'] = relu(mps0 @ W_fp + b_fp), chunked over b
            contri_s = cp.tile([128, NCH, DIM], BF, tag="contri_s")
            for j in range(NCH):
                cps = psB.tile([128, DIM], F32, tag="ps")
                nc.tensor.matmul(cps[:], lhsT=mps0T[:, 0, j * 128:(j + 1) * 128],
                                 rhs=wfp_s[:], start=True, stop=False)
                nc.tensor.matmul(cps[:], lhsT=onesbf_s[:], rhs=bfp_s[:],
                                 start=False, stop=True)
                nc.vector.tensor_scalar_max(contri_s[:, j, :], cps[:], 0.0)

            # mps_own[a, c] = mps0_own + A[own rows] @ contri   (dense)
            mps_ps = psB.tile([128, DIM], F32, tag="ps")
            for j in range(NCH):
                nc.tensor.matmul(mps_ps[:], lhsT=at_s[:, j, :],
                                 rhs=contri_s[:, j, :],
                                 start=(j == 0), stop=False)
            nc.tensor.matmul(mps_ps[:], lhsT=mps0oT[:, 0, :], rhs=idbf_s[:],
                             start=False, stop=True)
            mps_own = cp.tile([128, DIM], F32, tag="mps_own")
            nc.vector.tensor_copy(mps_own[:], mps_ps[:])

            # l2 normalize rows (free axis)
            nsq = smp.tile([128, 1], F32, tag="nsq")
            nscr = smp.tile([128, DIM], F32, tag="nscr")
            nc.scalar.activation(out=nscr[:], in_=mps_own[:],
                                 func=AFT.Square, accum_out=nsq[:])
            nrm = smp.tile([128, 1], F32, tag="nrm")
            nc.scalar.sqrt(nrm[:], nsq[:])
            nrm2 = smp.tile([128, 1], F32, tag="nrm2")
            nc.vector.tensor_scalar_max(nrm2[:], nrm[:], EPS)
            inv = smp.tile([128, 1], F32, tag="inv")
            nc.vector.reciprocal(inv[:], nrm2[:])
            mps_n = cp.tile([128, DIM], F32, tag="mps_n")  # [a_own, c]
            nc.vector.tensor_scalar_mul(mps_n[:], mps_own[:], inv[:])
            # transpose -> [c, a_own]
            mnt_ps = psB.tile([128, 128], F32, tag="ps")
            nc.tensor.transpose(mnt_ps[:], mps_n[:], idf_s[:])
            mps_nT = cp.tile([128, 128], F32, tag="mps_nT")
            nc.vector.tensor_copy(mps_nT[:], mnt_ps[:])

            # ---- main loop: mpo gather + ssq + diag matmuls ------------
            ssq = cp.tile([128, APC], F32, tag="ssq")   # [c, a_local]
            wT = cp.tile([128, APC], F32, tag="wT")     # [c, a_local]
            tmp_ps = psA.tile([128, N], F32, tag="tmp_ps")  # [c, b] accum

            nch_eff = int(stage[1:]) if stage.startswith("g") else NCH
            for ch in range(nch_eff):
                slab = slabp.tile([128, 1, ACH * N], BF, tag="slab")
                # 4 gather calls of 4096 idxs each (SWDGE ring carveout
                # holds ~1000 descriptors; 258/call leaves pipelining room)
                for q in range(GCALLS):
                    i0 = ch * (ACH * N // 16) + q * (GIDX // 16)
                    nc.gpsimd.dma_gather(
                        out_ap=slab[:, :, q * GIDX:(q + 1) * GIDX],
                        in_ap=e_bond[:],
                        idxs_ap=bidx_s[:, i0:i0 + GIDX // 16],
                        num_idxs=GIDX, num_idxs_reg=GIDX,
                        elem_size=DIM, transpose=True, single_packet=False)

                if stage == "gather":
                    gdump = smp.tile([128, GCALLS], BF, tag="gdump")
                    for q in range(GCALLS):
                        nc.vector.tensor_copy(gdump[:, q:q + 1],
                                              slab[:, 0, q * GIDX:q * GIDX + 1])
                    continue
                # ssq: first DVE_J rows on DVE (square + tree-add),
                # the rest on ACT (Square + accum_out).
                if DVE_J > 0:
                    sq = scrp.tile([128, DVE_J, N], BF, tag="dscr")
                    nc.vector.tensor_mul(
                        sq.rearrange("p j n -> p (j n)"),
                        slab[:, 0, :DVE_J * N], slab[:, 0, :DVE_J * N])
                    # tree-add over b within each row
                    t1 = scrp.tile([128, DVE_J, N // 2], BF, tag="tr1")
                    t2 = scrp.tile([128, DVE_J, N // 4], BF, tag="tr2")
                    nc.vector.tensor_add(t1[:], sq[:, :, :N // 2],
                                         sq[:, :, N // 2:])
                    nc.vector.tensor_add(t2[:], t1[:, :, :N // 4],
                                         t1[:, :, N // 4:])
                    lvls = [t2]
                    w_ = N // 4
                    while w_ > 2:
                        w_ //= 2
                        nxt = scrp.tile([128, DVE_J, w_], BF,
                                        tag=f"tr{w_}")
                        nc.vector.tensor_add(nxt[:], lvls[-1][:, :, :w_],
                                             lvls[-1][:, :, w_:])
                        lvls.append(nxt)
                    # final level -> f32 ssq columns
                    last = lvls[-1]
                    nc.vector.tensor_add(
                        ssq[:, ch * ACH:ch * ACH + DVE_J],
                        last[:, :, 0], last[:, :, 1])
                for j in range(DVE_J, ACH):
                    al = ch * ACH + j
                    scr = scrp.tile([128, N], BF, tag="ascr")
                    nc.scalar.activation(
                        out=scr[:], in_=slab[:, 0, j * N:(j + 1) * N],
                        func=AFT.Square, accum_out=ssq[:, al:al + 1])

                if stage == "ssq":
                    continue
                # w for this chunk: w[c, a] = mps_nT / max(sqrt(ssq), eps)
                c0, c1 = ch * ACH, (ch + 1) * ACH
                st = smp.tile([128, ACH], F32, tag="st")
                nc.scalar.sqrt(st[:], ssq[:, c0:c1])
                st2 = smp.tile([128, ACH], F32, tag="st2")
                nc.vector.tensor_scalar_max(st2[:], st[:], EPS)
                sti = smp.tile([128, ACH], F32, tag="sti")
                nc.vector.reciprocal(sti[:], st2[:])
                nc.vector.tensor_mul(wT[:, c0:c1], mps_nT[:, c0:c1], sti[:])

                # tmp[c, b] += diag(w_a) @ slab_a
                for j in range(ACH):
                    al = ch * ACH + j
                    diag = diagp.tile([128, 128], BF, tag="diag")
                    nc.vector.tensor_scalar_mul(diag[:], idbf_s[:],
                                                wT[:, al:al + 1])
                    nc.tensor.matmul(
                        tmp_ps[:, 0:512], lhsT=diag[:],
                        rhs=slab[:, 0, j * N:j * N + 512],
                        start=(al == 0), stop=(al == nch_eff * ACH - 1),
                        skip_group_check=True)
                    nc.tensor.matmul(
                        tmp_ps[:, 512:1024], lhsT=diag[:],
                        rhs=slab[:, 0, j * N + 512:(j + 1) * N],
                        start=(al == 0), stop=(al == nch_eff * ACH - 1),
                        skip_group_check=True)

            if stage in ("gather", "ssq", "mm"):  # early-exit debug stages
                # debug early-exit: emit a token output and stop
                dbg = smp.tile([MPC, 1], F32, tag="dbg")
                if stage == "gather":
                    nc.vector.tensor_copy(dbg[:], slab[:MPC, 0, 0:1])
                elif stage == "ssq":
                    nc.vector.tensor_copy(dbg[:], ssq[:MPC, 0:1])
                else:
                    tmp_dbg = cp.tile([128, N], F32, tag="tmp_dbg")
                    nc.vector.tensor_copy(tmp_dbg[:], tmp_ps[:])
                    nc.vector.tensor_copy(dbg[:], tmp_dbg[:MPC, 0:1])
                nc.sync.dma_start(out[:], dbg[:])
                nc.compile()
                return nc

            # ---- tmp -> b-major -> ReduceScatter -----------------------
            tmp_s = cp.tile([128, N], F32, tag="tmp_s")
            nc.vector.tensor_copy(tmp_s[:], tmp_ps[:])
            for j in range(NCH):
                tps = psB.tile([128, 128], F32, tag="ps")
                nc.tensor.transpose(tps[:], tmp_s[:, j * 128:(j + 1) * 128],
                                    idf_s[:])
                tts = smp.tile([128, 128], F32, tag="tts")
                nc.vector.tensor_copy(tts[:], tps[:])
                nc.sync.dma_start(cc_in[j * 128:(j + 1) * 128, :], tts[:])

            nc.gpsimd.collective_compute(
                "ReduceScatter", AOT.add,
                replica_groups=[list(range(NCORES))],
                ins=[cc_in[:]], outs=[cc_out[:]])

            # ---- finale: tn + MLP (own 8 molecules) --------------------
            tro = smp.tile([128, DIM], F32, tag="tro")  # [b_own, c]
            nc.sync.dma_start(tro[:], cc_out[:])
            prod = smp.tile([128, DIM], F32, tag="prod")
            nc.vector.tensor_mul(prod[:], tro[:], mps_n[:])
            tn_ps = psB.tile([MPC, DIM], F32, tag="ps")
            nc.tensor.matmul(tn_ps[:], lhsT=sel_s[:], rhs=prod[:],
                             start=True, stop=True)
            tn_s = smp.tile([MPC, DIM], F32, tag="tn_s")
            nc.vector.tensor_copy(tn_s[:], tn_ps[:])
            tnT_ps = psB.tile([128, MPC], F32, tag="ps")
            nc.tensor.transpose(tnT_ps[:], tn_s[:], idf_s[:MPC, :MPC])
            tnT_s = smp.tile([128, MPC], F32, tag="tnT_s")
            nc.vector.tensor_copy(tnT_s[:], tnT_ps[:])

            x0_ps = psB.tile([MPC, DIM], F32, tag="ps")
            nc.tensor.matmul(x0_ps[:], lhsT=tnT_s[:], rhs=w0_s[:],
                             start=True, stop=False)
            nc.tensor.matmul(x0_ps[:], lhsT=onesf_s[:, :MPC], rhs=b0_s[:],
                             start=False, stop=True)
            x0_s = smp.tile([MPC, DIM], F32, tag="x0_s")
            nc.vector.tensor_scalar_max(x0_s[:], x0_ps[:], 0.0)
            x0T_ps = psB.tile([128, MPC], F32, tag="ps")
            nc.tensor.transpose(x0T_ps[:], x0_s[:], idf_s[:MPC, :MPC])
            x0T_s = smp.tile([128, MPC], F32, tag="x0T_s")
            nc.vector.tensor_copy(x0T_s[:], x0T_ps[:])

            x1_ps = psB.tile([MPC, DIM], F32, tag="ps")
            nc.tensor.matmul(x1_ps[:], lhsT=x0T_s[:], rhs=w1_s[:],
                             start=True, stop=False)
            nc.tensor.matmul(x1_ps[:], lhsT=onesf_s[:, :MPC], rhs=b1_s[:],
                             start=False, stop=True)
            x1_s = smp.tile([MPC, DIM], F32, tag="x1_s")
            nc.vector.tensor_scalar_max(x1_s[:], x1_ps[:], 0.0)
            x1T_ps = psB.tile([128, MPC], F32, tag="ps")
            nc.tensor.transpose(x1T_ps[:], x1_s[:], idf_s[:MPC, :MPC])
            x1T_s = smp.tile([128, MPC], F32, tag="x1T_s")
            nc.vector.tensor_copy(x1T_s[:], x1T_ps[:])

            y_ps = psB.tile([MPC, 1], F32, tag="ps")
            nc.tensor.matmul(y_ps[:], lhsT=x1T_s[:], rhs=wp_s[:],
                             start=True, stop=False)
            nc.tensor.matmul(y_ps[:], lhsT=onesf_s[:, :MPC], rhs=bp_s[:, :1],
                             start=False, stop=True)
            y_s = smp.tile([MPC, 1], F32, tag="y_s")
            nc.vector.tensor_copy(y_s[:], y_ps[:])
            nc.sync.dma_start(out[:], y_s[:])

    nc.compile()
    return nc




def make_in_maps(fingerprints, adjacency, bond_index, E_fp, E_bond, W_fp,
                 b_fp, W_out0, b_out0, W_out1, b_out1, W_prop, b_prop):
    e_bond_bf = np.ascontiguousarray(E_bond.astype(BF16))
    e_fp_bf = np.ascontiguousarray(E_fp.astype(BF16))
    wfp_bf = np.ascontiguousarray(W_fp.astype(BF16))
    bfp_bf = b_fp.astype(BF16).reshape(1, DIM)
    w0_f = np.ascontiguousarray(W_out0.astype(np.float32))
    b0_f = b_out0.astype(np.float32).reshape(1, DIM)
    w1_f = np.ascontiguousarray(W_out1.astype(np.float32))
    b1_f = b_out1.astype(np.float32).reshape(1, DIM)
    wp_f = np.ascontiguousarray(W_prop.astype(np.float32))
    bp_f = b_prop.astype(np.float32).reshape(1, 1)
    idbf = np.eye(128, dtype=BF16)
    idf = np.eye(128, dtype=np.float32)
    onesbf = np.ones((1, 128), dtype=BF16)
    onesf = np.ones((1, 128), dtype=np.float32)
    # molecule-sum selector: sel[b, m] = 1 if b // 16 == m
    sel = np.zeros((128, MPC), dtype=np.float32)
    for m in range(MPC):
        sel[m * NA:(m + 1) * NA, m] = 1.0

    fpi_all = _wrap_idx16(fingerprints)

    in_maps = []
    for k in range(NCORES):
        rows = slice(k * APC, (k + 1) * APC)
        # bond idx: wrapped per gather call (each call wraps its own idxs)
        flat = bond_index[rows, :].astype(np.int16).reshape(
            NCH * GCALLS, GIDX)
        bidx = np.concatenate(
            [_wrap_idx16(flat[c]) for c in range(NCH * GCALLS)],
            axis=1)  # [128, 8192]
        at_k = np.ascontiguousarray(adjacency[rows, :].T.astype(BF16))
        fpi_own = _wrap_idx16(fingerprints[rows])
        in_maps.append({
            "e_bond": e_bond_bf, "e_fp": e_fp_bf,
            "fpi_all": fpi_all, "fpi_own": fpi_own, "bidx": bidx,
            "at": at_k, "wfp": wfp_bf, "bfp": bfp_bf,
            "w0": w0_f, "b0": b0_f, "w1": w1_f, "b1": b1_f,
            "wp": wp_f, "bp": bp_f,
            "idbf": idbf, "idf": idf, "onesbf": onesbf, "onesf": onesf,
            "sel": sel,
        })
    return in_maps


def make_in_maps_v4(fingerprints, adjacency, bond_index, E_fp, E_bond,
                    W_fp, b_fp, W_out0, b_out0, W_out1, b_out1, W_prop,
                    b_prop):
    e_bond_bf = np.ascontiguousarray(E_bond.astype(BF16))
    e_fp_bf = np.ascontiguousarray(E_fp.astype(BF16))
    wfp_bf = np.ascontiguousarray(W_fp.astype(BF16))
    bfp_bf = b_fp.astype(BF16).reshape(1, DIM)
    w0_f = np.ascontiguousarray(W_out0.astype(np.float32))
    b0T_f = b_out0.astype(np.float32).reshape(DIM, 1)
    w1_f = np.ascontiguousarray(W_out1.astype(np.float32))
    b1T_f = b_out1.astype(np.float32).reshape(DIM, 1)
    wp_f = np.ascontiguousarray(W_prop.astype(np.float32))
    bp_f = b_prop.astype(np.float32).reshape(1, 1)
    idbf = np.eye(128, dtype=BF16)
    idf = np.eye(128, dtype=np.float32)
    onesbf = np.ones((1, 128), dtype=BF16)
    onecol = np.ones((128, 1), dtype=BF16)
    selbf = np.zeros((128, MPC), dtype=BF16)
    for m in range(MPC):
        selbf[m * NA:(m + 1) * NA, m] = 1.0
    # T0 as the bf16-rounded row (matches the gathered slab precision)
    t0 = E_bond[0].astype(BF16).astype(np.float32)
    t0sqk = np.tile((float(N - NA) * t0 * t0).reshape(1, DIM), (128, 1))
    t0rep8 = np.tile(t0.reshape(1, DIM), (MPC, 1))
    t0sqk = np.ascontiguousarray(t0sqk.astype(np.float32))
    t0rep8 = np.ascontiguousarray(t0rep8.astype(np.float32))

    bond_index = np.asarray(bond_index)
    adjacency = np.asarray(adjacency)
    fingerprints = np.asarray(fingerprints)

    in_maps = []
    for k in range(NCORES):
        rows = np.arange(k * APC, (k + 1) * APC)
        # in-block bond indices, flat position n*128 + pa
        molg = k * MPC + np.arange(APC) // NA          # per own atom
        cols = molg[:, None] * NA + np.arange(NA)[None, :]   # [128, 16]
        blk = bond_index[rows[:, None], cols]          # [pa, n]
        flat = np.ascontiguousarray(blk.T).reshape(-1)  # [n*128 + pa]
        atT_k = np.ascontiguousarray(
            adjacency[np.ix_(rows, rows)].T.astype(BF16))
        in_maps.append({
            "e_bond": e_bond_bf, "e_fp": e_fp_bf,
            "fpi": _wrap_idx16(fingerprints[rows]),
            "ibx2": _wrap_idx16(flat),
            "atT": atT_k, "wfp": wfp_bf, "bfp": bfp_bf,
            "w0": w0_f, "b0T": b0T_f, "w1": w1_f, "b1T": b1T_f,
            "wp": wp_f, "bp": bp_f,
            "idbf": idbf, "idf": idf, "onesbf": onesbf, "onecol": onecol,
            "onecolf": np.ones((128, 1), dtype=np.float32),
            "selbf": selbf, "t0sqk": t0sqk, "t0rep8": t0rep8,
        })
    return in_maps


def _inputs_are_block_diag(adjacency, bond_index):
    mol = np.arange(N) // NA
    block = mol[:, None] == mol[None, :]
    off = ~block
    return bool(np.all(np.asarray(bond_index)[off] == 0)
                and np.all(np.asarray(adjacency)[off] == 0.0))


_NC_CACHE = {}


def _get_nc(key):
    if key not in _NC_CACHE:
        if key in ("rdma", "cc"):
            _NC_CACHE[key] = build_v4(comm=key)
        else:
            _NC_CACHE[key] = build_nc()
    return _NC_CACHE[key]


def run(inputs, trace=False, **kw):
    fast = _inputs_are_block_diag(inputs["adjacency"], inputs["bond_index"])
    if fast:
        comm = os.environ.get("BASS_GNN_COMM", "cc")
        nc = _get_nc(comm)
        in_maps = make_in_maps_v4(**inputs)
        res = run_bass_kernel_spmd(nc, in_maps,
                                   core_ids=list(range(NCORES)),
                                   trace=trace, **kw)
        out = np.concatenate(
            [res.results[k]["out"].reshape(MPC) for k in range(NCORES)],
            axis=0).reshape(B, 1).astype(np.float32)
        return out, res
    nc = _get_nc("full")
    in_maps = make_in_maps(**inputs)
    res = run_bass_kernel_spmd(nc, in_maps, core_ids=list(range(NCORES)),
                               trace=trace, **kw)
    out = np.concatenate([res.results[k]["out"] for k in range(NCORES)],
                         axis=0).astype(np.float32)
    return out, res


def kernel(**inputs):
    out, _ = run(inputs, trace=False)
    return out

